# revision 3
# baseline (speedup 1.0000x reference)
# BinarizeLinear on 8 Trainium2 NeuronCores.
#
# reference: out = binarize(x) @ binarize(weight).T + bias
#   x      [16384, 2048] f32
#   weight [2048, 2048]  f32
#   bias   [2048]        f32
#   out    [16384, 2048] f32
#
# Strategy (data-parallel over rows of x, weight/bias replicated):
#   - Each of the 8 cores gets a 2048-row shard of x.
#   - Host uploads x-shard and weight TRANSPOSED (K on the leading axis) as
#     bf16, so the contraction dim lands on SBUF partitions with a natural
#     contiguous DMA.  bf16 is exact for this problem: only sign(x) matters
#     downstream, and the f32->bf16 cast preserves sign for every |v| >=
#     2^-133 (randn values are nowhere near that).
#   - Device binarizes both operands to exactly +-1.0 bf16 with a single
#     tensor_scalar pass over a uint16 bitcast: (v & 0x8000) | 0x3F80.
#   - out.T[n, m] = sum_k wbT[k, n] * xbT[k, m] accumulates in PSUM over
#     16 K-tiles (matmul lhsT = wbT tile, rhs = xbT chunk).
#   - ScalarE evacuates PSUM with a fused per-partition bias add
#     (activation Identity, bias = bias[n] column), giving out.T + bias.
#   - Host transposes each core's out.T shard back and stacks.

import sys

import numpy as np

try:
    import concourse  # noqa: F401
except ImportError:
    sys.path.insert(0, "/opt/trn_rl_repo")

import ml_dtypes
from contextlib import ExitStack

import concourse.bass as bass
import concourse.mybir as mybir
import concourse.tile as tile
from concourse import bacc
from concourse.bass_utils import run_bass_kernel_spmd

NCORES = 8
K = 2048          # contraction dim (in_features)
NF = 2048         # out features
MTOT = 16384      # rows of x
MS = MTOT // NCORES  # rows per core
P = 128           # partitions
MC = 512          # moving free-dim chunk (one PSUM bank of f32)
KT = K // P       # 16 k-tiles
NT = NF // P      # 16 n-tiles
MT = MS // MC     # 4 m-chunks

BF16 = mybir.dt.bfloat16
F32 = mybir.dt.float32
U16 = mybir.dt.uint16


def build_nc(debug=False):
    nc = bacc.Bacc(
        "TRN2", target_bir_lowering=False, debug=debug, num_devices=NCORES
    )
    xT = nc.dram_tensor("xT", [K, MS], BF16, kind="ExternalInput").ap()
    wT = nc.dram_tensor("wT", [K, NF], BF16, kind="ExternalInput").ap()
    bias = nc.dram_tensor("bias", [NF], F32, kind="ExternalInput").ap()
    outT = nc.dram_tensor("outT", [NF, MS], F32, kind="ExternalOutput").ap()

    with tile.TileContext(nc) as tc:
        with ExitStack() as ctx:
            const = ctx.enter_context(tc.tile_pool(name="const", bufs=1))
            res = ctx.enter_context(tc.tile_pool(name="res", bufs=1))
            psum = ctx.enter_context(
                tc.tile_pool(name="ps", bufs=2, space=bass.MemorySpace.PSUM)
            )
            outp = ctx.enter_context(tc.tile_pool(name="out", bufs=3))

            # bias[n] laid out [128, 16]: column t holds bias[t*128:(t+1)*128]
            bias_t = const.tile([P, NT], F32)
            nc.sync.dma_start(
                out=bias_t[:], in_=bias.rearrange("(t p) -> p t", p=P)
            )

            def load_bin(dram, name, t):
                # DMA one [128, width] bf16 k-strip, binarize in place:
                # keep sign bit, force exponent/mantissa of 1.0.
                tl = res.tile([P, dram.shape[1]], BF16, tag=f"{name}{t}")
                nc.sync.dma_start(out=tl[:], in_=dram[t * P : (t + 1) * P, :])
                nc.vector.tensor_scalar(
                    tl[:].bitcast(U16),
                    tl[:].bitcast(U16),
                    0x8000,
                    0x3F80,
                    mybir.AluOpType.bitwise_and,
                    mybir.AluOpType.bitwise_or,
                )
                return tl

            wb = []
            xb = []
            for t in range(KT):
                wb.append(load_bin(wT, "w", t))
                xb.append(load_bin(xT, "x", t))

            for n in range(NT):
                pss = [
                    psum.tile([P, MC], F32, tag=f"ps{mc}", name=f"ps_n{n}_{mc}")
                    for mc in range(MT)
                ]
                for k in range(KT):
                    lhsT = wb[k][:, n * P : (n + 1) * P]
                    for mc in range(MT):
                        nc.tensor.matmul(
                            pss[mc][:],
                            lhsT,
                            xb[k][:, mc * MC : (mc + 1) * MC],
                            start=(k == 0),
                            stop=(k == KT - 1),
                        )
                ot = outp.tile([P, MS], F32, tag="o")
                for mc in range(MT):
                    nc.scalar.activation(
                        ot[:, mc * MC : (mc + 1) * MC],
                        pss[mc][:],
                        mybir.ActivationFunctionType.Identity,
                        bias=bias_t[:, n : n + 1],
                    )
                nc.sync.dma_start(
                    out=outT[n * P : (n + 1) * P, :], in_=ot[:]
                )

    nc.compile()
    return nc


_NC = None


def _get_nc():
    global _NC
    if _NC is None:
        _NC = build_nc()
    return _NC


def _to_bf16_T(a):
    # Transposed bf16 copy with +0.0 canonicalized to -0.0.  The device
    # binarize keys on the sign bit, and reference binarize maps 0 -> -1;
    # +-0 are the same numeric value, so this re-encoding is lossless and
    # makes the sign-bit semantics match the reference exactly.
    b = np.ascontiguousarray(a.astype(ml_dtypes.bfloat16).T)
    u = b.view(np.uint16)
    u[u == 0] = 0x8000
    return b


def make_in_maps(x, weight, bias):
    x = np.asarray(x, dtype=np.float32)
    weight = np.asarray(weight, dtype=np.float32)
    bias = np.asarray(bias, dtype=np.float32)
    wTb = _to_bf16_T(weight)
    in_maps = []
    for i in range(NCORES):
        xTb = _to_bf16_T(x[i * MS : (i + 1) * MS, :])
        in_maps.append({"xT": xTb, "wT": wTb, "bias": bias})
    return in_maps


def assemble_out(results):
    out = np.empty((MTOT, NF), dtype=np.float32)
    for i in range(NCORES):
        out[i * MS : (i + 1) * MS, :] = results[i]["outT"].T
    return out


def run(x, weight, bias, trace=False, **kwargs):
    nc = _get_nc()
    in_maps = make_in_maps(x, weight, bias)
    res = run_bass_kernel_spmd(
        nc, in_maps, list(range(NCORES)), trace=trace, **kwargs
    )
    return assemble_out(res.results), res


def kernel(x, weight, bias):
    out, _ = run(x, weight, bias)
    return out


# revision 4
# speedup vs baseline: 1.7014x; 1.7014x over previous
# BinarizeLinear on 8 Trainium2 NeuronCores.
#
# reference: out = binarize(x) @ binarize(weight).T + bias
#   x      [16384, 2048] f32
#   weight [2048, 2048]  f32
#   bias   [2048]        f32
#   out    [16384, 2048] f32
#
# Strategy (data-parallel over rows of x, weight/bias replicated):
#   - Each of the 8 cores gets a 2048-row shard of x.
#   - Host uploads x-shard and weight TRANSPOSED (K on the leading axis) so
#     the contraction dim lands on SBUF partitions with a natural contiguous
#     DMA.  Uploads are fp8e4m3 with magnitudes clipped into fp8 range and
#     zeros encoded as tiny negatives: a lossless SIGN encoding, which is the
#     only thing binarize consumes (reference maps 0 -> -1, hence -0 style
#     encoding for zeros).
#   - Device binarizes both operands to exactly +-1.0 with a single
#     tensor_scalar pass over a uint8 bitcast: (v & 0x80) | 0x38.
#   - out.T[n, m] = sum_k wbT[k, n] * xbT[k, m] accumulates in PSUM with
#     DoubleRow fp8 matmuls (2 MACs/cell/cycle, contraction 256 per MM).
#   - ScalarE evacuates PSUM with a fused per-partition bias add
#     (activation Identity, bias = bias[n] column), giving out.T + bias.
#   - Host transposes each core's out.T shard back and stacks.

import sys

import numpy as np

try:
    import concourse  # noqa: F401
except ImportError:
    sys.path.insert(0, "/opt/trn_rl_repo")

import ml_dtypes
from contextlib import ExitStack

import concourse.bass as bass
import concourse.mybir as mybir
import concourse.tile as tile
from concourse import bacc
from concourse.bass_utils import run_bass_kernel_spmd

NCORES = 8
K = 2048          # contraction dim (in_features)
NF = 2048         # out features
MTOT = 16384      # rows of x
MS = MTOT // NCORES  # rows per core
P = 128           # partitions
MC = 512          # moving free-dim chunk (one PSUM bank of f32)
KT2 = K // (2 * P)   # 8 double-k-tiles (DoubleRow contracts 256/MM)
NT = NF // P      # 16 n-tiles
MT = MS // MC     # 4 m-chunks

F32 = mybir.dt.float32
FP8 = mybir.dt.float8e4
U8 = mybir.dt.uint8


def build_nc(debug=False):
    nc = bacc.Bacc(
        "TRN2", target_bir_lowering=False, debug=debug, num_devices=NCORES
    )
    xT = nc.dram_tensor("xT", [K, MS], FP8, kind="ExternalInput").ap()
    wT = nc.dram_tensor("wT", [K, NF], FP8, kind="ExternalInput").ap()
    bias = nc.dram_tensor("bias", [NF], F32, kind="ExternalInput").ap()
    outT = nc.dram_tensor("outT", [NF, MS], F32, kind="ExternalOutput").ap()

    with tile.TileContext(nc) as tc:
        with ExitStack() as ctx:
            const = ctx.enter_context(tc.tile_pool(name="const", bufs=1))
            res = ctx.enter_context(tc.tile_pool(name="res", bufs=1))
            psum = ctx.enter_context(
                tc.tile_pool(name="ps", bufs=2, space=bass.MemorySpace.PSUM)
            )
            outp = ctx.enter_context(tc.tile_pool(name="out", bufs=3))

            # bias[n] laid out [128, 16]: column t holds bias[t*128:(t+1)*128]
            bias_t = const.tile([P, NT], F32)
            nc.sync.dma_start(
                out=bias_t[:], in_=bias.rearrange("(t p) -> p t", p=P)
            )

            def load_bin(dram, name, t):
                # [128, 2, width] fp8: [k', j, c] = dramT[(2t+j)*128+k', c].
                # DoubleRow matmul contracts over (partition, j).
                # Binarize in place: keep sign bit, force the rest to 1.0.
                width = dram.shape[1]
                tl = res.tile([P, 2, width], FP8, tag=f"{name}{t}")
                nc.sync.dma_start(
                    out=tl[:],
                    in_=dram[2 * t * P : (2 * t + 2) * P, :].rearrange(
                        "(j p) c -> p j c", j=2
                    ),
                )
                nc.vector.tensor_scalar(
                    tl[:].bitcast(U8),
                    tl[:].bitcast(U8),
                    0x80,
                    0x38,
                    mybir.AluOpType.bitwise_and,
                    mybir.AluOpType.bitwise_or,
                )
                return tl

            wb = []
            xb = []
            for t in range(KT2):
                wb.append(load_bin(wT, "w", t))
                xb.append(load_bin(xT, "x", t))

            for n in range(NT):
                pss = [
                    psum.tile([P, MC], F32, tag=f"ps{mc}", name=f"ps_n{n}_{mc}")
                    for mc in range(MT)
                ]
                for t in range(KT2):
                    lhsT = wb[t][:, :, n * P : (n + 1) * P]
                    for mc in range(MT):
                        nc.tensor.matmul(
                            pss[mc][:],
                            lhsT,
                            xb[t][:, :, mc * MC : (mc + 1) * MC],
                            start=(t == 0),
                            stop=(t == KT2 - 1),
                            perf_mode=mybir.MatmulPerfMode.DoubleRow,
                        )
                ot = outp.tile([P, MS], F32, tag="o")
                for mc in range(MT):
                    nc.scalar.activation(
                        ot[:, mc * MC : (mc + 1) * MC],
                        pss[mc][:],
                        mybir.ActivationFunctionType.Identity,
                        bias=bias_t[:, n : n + 1],
                    )
                nc.sync.dma_start(
                    out=outT[n * P : (n + 1) * P, :], in_=ot[:]
                )

    nc.compile()
    return nc


_NC = None


def _get_nc():
    global _NC
    if _NC is None:
        _NC = build_nc()
    return _NC


def _to_fp8_T(a):
    # Transposed fp8 copy preserving the SIGN of every element exactly
    # (magnitudes are irrelevant downstream -- the device binarizes).
    # Magnitudes are clipped into e4m3 range so the cast can't flush to
    # zero or overflow, and exact zeros are encoded as tiny NEGATIVES
    # because reference binarize maps 0 -> -1.
    at = a.T
    mag = np.clip(np.abs(at), 0.002, 240.0)
    enc = np.where(at > 0, mag, -mag).astype(ml_dtypes.float8_e4m3fn)
    return np.ascontiguousarray(enc)


def make_in_maps(x, weight, bias):
    x = np.asarray(x, dtype=np.float32)
    weight = np.asarray(weight, dtype=np.float32)
    bias = np.asarray(bias, dtype=np.float32)
    wTb = _to_fp8_T(weight)
    in_maps = []
    for i in range(NCORES):
        xTb = _to_fp8_T(x[i * MS : (i + 1) * MS, :])
        in_maps.append({"xT": xTb, "wT": wTb, "bias": bias})
    return in_maps


def assemble_out(results):
    out = np.empty((MTOT, NF), dtype=np.float32)
    for i in range(NCORES):
        out[i * MS : (i + 1) * MS, :] = results[i]["outT"].T
    return out


def run(x, weight, bias, trace=False, **kwargs):
    nc = _get_nc()
    in_maps = make_in_maps(x, weight, bias)
    res = run_bass_kernel_spmd(
        nc, in_maps, list(range(NCORES)), trace=trace, **kwargs
    )
    return assemble_out(res.results), res


def kernel(x, weight, bias):
    out, _ = run(x, weight, bias)
    return out


# revision 7
# speedup vs baseline: 1.8743x; 1.1016x over previous
# BinarizeLinear on 8 Trainium2 NeuronCores.
#
# reference: out = binarize(x) @ binarize(weight).T + bias
#   x      [16384, 2048] f32
#   weight [2048, 2048]  f32
#   bias   [2048]        f32
#   out    [16384, 2048] f32
#
# Strategy (data-parallel over rows of x, weight/bias replicated):
#   - Each of the 8 cores gets a 2048-row shard of x.
#   - Host uploads x-shard and weight TRANSPOSED (K on the leading axis) so
#     the contraction dim lands on SBUF partitions with a natural contiguous
#     DMA.  Uploads are fp8e4m3 with magnitudes clipped into fp8 range and
#     zeros encoded as tiny negatives: a lossless SIGN encoding, which is the
#     only thing binarize consumes (reference maps 0 -> -1, hence -0 style
#     encoding for zeros).
#   - Device binarizes both operands to exactly +-1.0 with a single
#     tensor_scalar pass over a uint8 bitcast: (v & 0x80) | 0x38.
#   - out.T[n, m] = sum_k wbT[k, n] * xbT[k, m] accumulates in PSUM with
#     DoubleRow fp8 matmuls (2 MACs/cell/cycle, contraction 256 per MM).
#   - ScalarE evacuates PSUM with a fused per-partition bias add
#     (activation Identity, bias = bias[n] column), giving out.T + bias.
#   - Host transposes each core's out.T shard back and stacks.

import sys

import numpy as np

try:
    import concourse  # noqa: F401
except ImportError:
    sys.path.insert(0, "/opt/trn_rl_repo")

import ml_dtypes
from contextlib import ExitStack

import concourse.bass as bass
import concourse.mybir as mybir
import concourse.tile as tile
from concourse import bacc
from concourse.bass_utils import run_bass_kernel_spmd

NCORES = 8
K = 2048          # contraction dim (in_features)
NF = 2048         # out features
MTOT = 16384      # rows of x
MS = MTOT // NCORES  # rows per core
P = 128           # partitions
MC = 512          # moving free-dim chunk (one PSUM bank of f32)
KT2 = K // (2 * P)   # 8 double-k-tiles (DoubleRow contracts 256/MM)
NT = NF // P      # 16 n-tiles
MT = MS // MC     # 4 m-chunks

F32 = mybir.dt.float32
FP8 = mybir.dt.float8e4
U8 = mybir.dt.uint8


def build_nc(debug=False):
    nc = bacc.Bacc(
        "TRN2", target_bir_lowering=False, debug=debug, num_devices=NCORES
    )
    xT = nc.dram_tensor("xT", [K, MS], FP8, kind="ExternalInput").ap()
    wT = nc.dram_tensor("wT", [K, NF], FP8, kind="ExternalInput").ap()
    bias = nc.dram_tensor("bias", [NF], F32, kind="ExternalInput").ap()
    outT = nc.dram_tensor("outT", [NF, MS], F32, kind="ExternalOutput").ap()

    U16 = mybir.dt.uint16
    NG = 2  # n-tiles per group; NG*MT psum banks live at once

    with tile.TileContext(nc) as tc:
        with ExitStack() as ctx:
            const = ctx.enter_context(tc.tile_pool(name="const", bufs=1))
            res = ctx.enter_context(tc.tile_pool(name="res", bufs=1))
            psum = ctx.enter_context(
                tc.tile_pool(name="ps", bufs=1, space=bass.MemorySpace.PSUM)
            )
            outp = ctx.enter_context(tc.tile_pool(name="out", bufs=3))

            # bias[n] laid out [128, 16]: column t holds bias[t*128:(t+1)*128]
            bias_t = const.tile([P, NT], F32)
            nc.gpsimd.dma_start(
                out=bias_t[:], in_=bias.rearrange("(t p) -> p t", p=P)
            )

            def load_bin(dram, name, t, dma_eng):
                # [128, 2, width] fp8: [k', j, c] = dramT[(2t+j)*128+k', c].
                # DoubleRow matmul contracts over (partition, j).
                # Binarize in place: keep sign bit, force the rest to 1.0.
                # The pass runs on a uint16 view (two fp8 per ALU element).
                width = dram.shape[1]
                tl = res.tile([P, 2, width], FP8, tag=f"{name}{t}")
                dma_eng.dma_start(
                    out=tl[:],
                    in_=dram[2 * t * P : (2 * t + 2) * P, :].rearrange(
                        "(j p) c -> p j c", j=2
                    ),
                )
                nc.vector.tensor_scalar(
                    tl[:].bitcast(U16),
                    tl[:].bitcast(U16),
                    0x8080,
                    0x3838,
                    mybir.AluOpType.bitwise_and,
                    mybir.AluOpType.bitwise_or,
                )
                return tl

            wb = []
            xb = []
            for t in range(KT2):
                wb.append(load_bin(wT, "w", t, nc.sync))
                xb.append(load_bin(xT, "x", t, nc.scalar))

            for g in range(NT // NG):
                pss = [
                    [
                        psum.tile(
                            [P, MC], F32, tag=f"ps{i}_{mc}", name=f"ps_{g}_{i}_{mc}"
                        )
                        for mc in range(MT)
                    ]
                    for i in range(NG)
                ]
                for t in range(KT2):
                    for i in range(NG):
                        n = g * NG + i
                        lhsT = wb[t][:, :, n * P : (n + 1) * P]
                        for mc in range(MT):
                            nc.tensor.matmul(
                                pss[i][mc][:],
                                lhsT,
                                xb[t][:, :, mc * MC : (mc + 1) * MC],
                                start=(t == 0),
                                stop=(t == KT2 - 1),
                                perf_mode=mybir.MatmulPerfMode.DoubleRow,
                            )
                for i in range(NG):
                    n = g * NG + i
                    ot = outp.tile([P, MS], F32, tag=f"o{i}", name=f"o_{g}_{i}")
                    for mc in range(MT):
                        nc.scalar.activation(
                            ot[:, mc * MC : (mc + 1) * MC],
                            pss[i][mc][:],
                            mybir.ActivationFunctionType.Identity,
                            bias=bias_t[:, n : n + 1],
                        )
                        nc.sync.dma_start(
                            out=outT[n * P : (n + 1) * P, mc * MC : (mc + 1) * MC],
                            in_=ot[:, mc * MC : (mc + 1) * MC],
                        )

    nc.compile()
    return nc


_NC = None


def _get_nc():
    global _NC
    if _NC is None:
        _NC = build_nc()
    return _NC


def _to_fp8_T(a):
    # Transposed fp8 copy preserving the SIGN of every element exactly
    # (magnitudes are irrelevant downstream -- the device binarizes).
    # Magnitudes are clipped into e4m3 range so the cast can't flush to
    # zero or overflow, and exact zeros are encoded as tiny NEGATIVES
    # because reference binarize maps 0 -> -1.
    at = a.T
    mag = np.clip(np.abs(at), 0.002, 240.0)
    enc = np.where(at > 0, mag, -mag).astype(ml_dtypes.float8_e4m3fn)
    return np.ascontiguousarray(enc)


def make_in_maps(x, weight, bias):
    x = np.asarray(x, dtype=np.float32)
    weight = np.asarray(weight, dtype=np.float32)
    bias = np.asarray(bias, dtype=np.float32)
    wTb = _to_fp8_T(weight)
    in_maps = []
    for i in range(NCORES):
        xTb = _to_fp8_T(x[i * MS : (i + 1) * MS, :])
        in_maps.append({"xT": xTb, "wT": wTb, "bias": bias})
    return in_maps


def assemble_out(results):
    out = np.empty((MTOT, NF), dtype=np.float32)
    for i in range(NCORES):
        out[i * MS : (i + 1) * MS, :] = results[i]["outT"].T
    return out


def run(x, weight, bias, trace=False, **kwargs):
    nc = _get_nc()
    in_maps = make_in_maps(x, weight, bias)
    res = run_bass_kernel_spmd(
        nc, in_maps, list(range(NCORES)), trace=trace, **kwargs
    )
    return assemble_out(res.results), res


def kernel(x, weight, bias):
    out, _ = run(x, weight, bias)
    return out


# revision 8
# speedup vs baseline: 1.9006x; 1.0140x over previous
# BinarizeLinear on 8 Trainium2 NeuronCores.
#
# reference: out = binarize(x) @ binarize(weight).T + bias
#   x      [16384, 2048] f32
#   weight [2048, 2048]  f32
#   bias   [2048]        f32
#   out    [16384, 2048] f32
#
# Strategy (data-parallel over rows of x, weight/bias replicated):
#   - Each of the 8 cores gets a 2048-row shard of x.
#   - Host uploads x-shard and weight TRANSPOSED (K on the leading axis) so
#     the contraction dim lands on SBUF partitions with a natural contiguous
#     DMA.  Uploads are fp8e4m3 with magnitudes clipped into fp8 range and
#     zeros encoded as tiny negatives: a lossless SIGN encoding, which is the
#     only thing binarize consumes (reference maps 0 -> -1, hence -0 style
#     encoding for zeros).
#   - Device binarizes both operands to exactly +-1.0 with a single
#     tensor_scalar pass over a uint8 bitcast: (v & 0x80) | 0x38.
#   - out.T[n, m] = sum_k wbT[k, n] * xbT[k, m] accumulates in PSUM with
#     DoubleRow fp8 matmuls (2 MACs/cell/cycle, contraction 256 per MM).
#   - ScalarE evacuates PSUM with a fused per-partition bias add
#     (activation Identity, bias = bias[n] column), giving out.T + bias.
#   - Host transposes each core's out.T shard back and stacks.

import sys

import numpy as np

try:
    import concourse  # noqa: F401
except ImportError:
    sys.path.insert(0, "/opt/trn_rl_repo")

import ml_dtypes
from contextlib import ExitStack

import concourse.bass as bass
import concourse.mybir as mybir
import concourse.tile as tile
from concourse import bacc
from concourse.bass_utils import run_bass_kernel_spmd

NCORES = 8
K = 2048          # contraction dim (in_features)
NF = 2048         # out features
MTOT = 16384      # rows of x
MS = MTOT // NCORES  # rows per core
P = 128           # partitions
MC = 512          # moving free-dim chunk (one PSUM bank of f32)
KT2 = K // (2 * P)   # 8 double-k-tiles (DoubleRow contracts 256/MM)
NT = NF // P      # 16 n-tiles
MT = MS // MC     # 4 m-chunks

F32 = mybir.dt.float32
FP8 = mybir.dt.float8e4
U8 = mybir.dt.uint8


def build_nc(debug=False):
    nc = bacc.Bacc(
        "TRN2", target_bir_lowering=False, debug=debug, num_devices=NCORES
    )
    xT = nc.dram_tensor("xT", [K, MS], FP8, kind="ExternalInput").ap()
    wT = nc.dram_tensor("wT", [K, NF], FP8, kind="ExternalInput").ap()
    bias = nc.dram_tensor("bias", [NF], F32, kind="ExternalInput").ap()
    outT = nc.dram_tensor("outT", [NF, MS], F32, kind="ExternalOutput").ap()

    U16 = mybir.dt.uint16
    NG = 2  # n-tiles per group; NG*MT psum banks live at once

    with tile.TileContext(nc) as tc:
        with ExitStack() as ctx:
            const = ctx.enter_context(tc.tile_pool(name="const", bufs=1))
            res = ctx.enter_context(tc.tile_pool(name="res", bufs=1))
            psum = ctx.enter_context(
                tc.tile_pool(name="ps", bufs=1, space=bass.MemorySpace.PSUM)
            )
            outp = ctx.enter_context(tc.tile_pool(name="out", bufs=3))

            # bias[n] laid out [128, 16]: column t holds bias[t*128:(t+1)*128]
            bias_t = const.tile([P, NT], F32)
            nc.gpsimd.dma_start(
                out=bias_t[:], in_=bias.rearrange("(t p) -> p t", p=P)
            )

            HW = NF // 2  # half-strip width

            def load_bin(dram, name, t, h, dma_eng):
                # [128, 2, HW] fp8: [k', j, c] = dramT[(2t+j)*128+h*HW+c].
                # DoubleRow matmul contracts over (partition, j).
                # Binarize in place: keep sign bit, force the rest to 1.0.
                # The pass runs on a uint16 view (two fp8 per ALU element).
                tl = res.tile([P, 2, HW], FP8, tag=f"{name}{t}_{h}")
                dma_eng.dma_start(
                    out=tl[:],
                    in_=dram[
                        2 * t * P : (2 * t + 2) * P, h * HW : (h + 1) * HW
                    ].rearrange("(j p) c -> p j c", j=2),
                )
                nc.vector.tensor_scalar(
                    tl[:].bitcast(U16),
                    tl[:].bitcast(U16),
                    0x8080,
                    0x3838,
                    mybir.AluOpType.bitwise_and,
                    mybir.AluOpType.bitwise_or,
                )
                return tl

            wb = []
            xb = []
            for t in range(KT2):
                wb.append(
                    [
                        load_bin(wT, "w", t, 0, nc.sync),
                        load_bin(wT, "w", t, 1, nc.sync),
                    ]
                )
                xb.append(
                    [
                        load_bin(xT, "x", t, 0, nc.scalar),
                        load_bin(xT, "x", t, 1, nc.scalar),
                    ]
                )

            NPH = HW // P   # n-tiles per w half
            MCH = HW // MC  # m-chunks per x half

            def w_slice(t, n):
                return wb[t][n // NPH][
                    :, :, (n % NPH) * P : (n % NPH + 1) * P
                ]

            def x_slice(t, mc):
                return xb[t][mc // MCH][
                    :, :, (mc % MCH) * MC : (mc % MCH + 1) * MC
                ]

            NGRP = NT // NG
            for g in range(NGRP):
                pss = [
                    [
                        psum.tile(
                            [P, MC], F32, tag=f"ps{i}_{mc}", name=f"ps_{g}_{i}_{mc}"
                        )
                        for mc in range(MT)
                    ]
                    for i in range(NG)
                ]
                ots = [
                    outp.tile([P, MS], F32, tag=f"o{i}", name=f"o_{g}_{i}")
                    for i in range(NG)
                ]

                def evacuate(i, mc):
                    n = g * NG + i
                    nc.scalar.activation(
                        ots[i][:, mc * MC : (mc + 1) * MC],
                        pss[i][mc][:],
                        mybir.ActivationFunctionType.Identity,
                        bias=bias_t[:, n : n + 1],
                    )
                    nc.sync.dma_start(
                        out=outT[n * P : (n + 1) * P, mc * MC : (mc + 1) * MC],
                        in_=ots[i][:, mc * MC : (mc + 1) * MC],
                    )

                if g < NGRP - 1:
                    # k-tile outer: consume input strips as they stream in.
                    for t in range(KT2):
                        for i in range(NG):
                            for mc in range(MT):
                                nc.tensor.matmul(
                                    pss[i][mc][:],
                                    w_slice(t, g * NG + i),
                                    x_slice(t, mc),
                                    start=(t == 0),
                                    stop=(t == KT2 - 1),
                                    perf_mode=mybir.MatmulPerfMode.DoubleRow,
                                )
                    for i in range(NG):
                        for mc in range(MT):
                            evacuate(i, mc)
                else:
                    # Last group: bank-major so evacuation and output DMA of
                    # bank b overlap the matmuls of bank b+1 (shrinks the
                    # kernel tail to one bank's epilogue).
                    for i in range(NG):
                        for mc in range(MT):
                            for t in range(KT2):
                                nc.tensor.matmul(
                                    pss[i][mc][:],
                                    w_slice(t, g * NG + i),
                                    x_slice(t, mc),
                                    start=(t == 0),
                                    stop=(t == KT2 - 1),
                                    perf_mode=mybir.MatmulPerfMode.DoubleRow,
                                )
                            evacuate(i, mc)

    nc.compile()
    return nc


_NC = None


def _get_nc():
    global _NC
    if _NC is None:
        _NC = build_nc()
    return _NC


def _to_fp8_T(a):
    # Transposed fp8 copy preserving the SIGN of every element exactly
    # (magnitudes are irrelevant downstream -- the device binarizes).
    # Magnitudes are clipped into e4m3 range so the cast can't flush to
    # zero or overflow, and exact zeros are encoded as tiny NEGATIVES
    # because reference binarize maps 0 -> -1.
    at = a.T
    mag = np.clip(np.abs(at), 0.002, 240.0)
    enc = np.where(at > 0, mag, -mag).astype(ml_dtypes.float8_e4m3fn)
    return np.ascontiguousarray(enc)


def make_in_maps(x, weight, bias):
    x = np.asarray(x, dtype=np.float32)
    weight = np.asarray(weight, dtype=np.float32)
    bias = np.asarray(bias, dtype=np.float32)
    wTb = _to_fp8_T(weight)
    in_maps = []
    for i in range(NCORES):
        xTb = _to_fp8_T(x[i * MS : (i + 1) * MS, :])
        in_maps.append({"xT": xTb, "wT": wTb, "bias": bias})
    return in_maps


def assemble_out(results):
    out = np.empty((MTOT, NF), dtype=np.float32)
    for i in range(NCORES):
        out[i * MS : (i + 1) * MS, :] = results[i]["outT"].T
    return out


def run(x, weight, bias, trace=False, **kwargs):
    nc = _get_nc()
    in_maps = make_in_maps(x, weight, bias)
    res = run_bass_kernel_spmd(
        nc, in_maps, list(range(NCORES)), trace=trace, **kwargs
    )
    return assemble_out(res.results), res


def kernel(x, weight, bias):
    out, _ = run(x, weight, bias)
    return out


# revision 11
# speedup vs baseline: 1.9180x; 1.0092x over previous
# BinarizeLinear on 8 Trainium2 NeuronCores.
#
# reference: out = binarize(x) @ binarize(weight).T + bias
#   x      [16384, 2048] f32
#   weight [2048, 2048]  f32
#   bias   [2048]        f32
#   out    [16384, 2048] f32
#
# Strategy (data-parallel over rows of x, weight/bias replicated):
#   - Each of the 8 cores gets a 2048-row shard of x.
#   - Host uploads x-shard and weight TRANSPOSED (K on the leading axis) so
#     the contraction dim lands on SBUF partitions with a natural contiguous
#     DMA.  Uploads are fp8e4m3 with magnitudes clipped into fp8 range and
#     zeros encoded as tiny negatives: a lossless SIGN encoding, which is the
#     only thing binarize consumes (reference maps 0 -> -1, hence -0 style
#     encoding for zeros).
#   - Device binarizes both operands to exactly +-1.0 with a single
#     tensor_scalar pass over a uint8 bitcast: (v & 0x80) | 0x38.
#   - out.T[n, m] = sum_k wbT[k, n] * xbT[k, m] accumulates in PSUM with
#     DoubleRow fp8 matmuls (2 MACs/cell/cycle, contraction 256 per MM).
#   - ScalarE evacuates PSUM with a fused per-partition bias add
#     (activation Identity, bias = bias[n] column), giving out.T + bias.
#   - Host transposes each core's out.T shard back and stacks.

import sys

import numpy as np

try:
    import concourse  # noqa: F401
except ImportError:
    sys.path.insert(0, "/opt/trn_rl_repo")

import ml_dtypes
from contextlib import ExitStack

import concourse.bass as bass
import concourse.mybir as mybir
import concourse.tile as tile
from concourse import bacc
from concourse.bass_utils import run_bass_kernel_spmd

NCORES = 8
K = 2048          # contraction dim (in_features)
NF = 2048         # out features
MTOT = 16384      # rows of x
MS = MTOT // NCORES  # rows per core
P = 128           # partitions
MC = 512          # moving free-dim chunk (one PSUM bank of f32)
KT2 = K // (2 * P)   # 8 double-k-tiles (DoubleRow contracts 256/MM)
NT = NF // P      # 16 n-tiles
MT = MS // MC     # 4 m-chunks

F32 = mybir.dt.float32
FP8 = mybir.dt.float8e4
U8 = mybir.dt.uint8


def build_nc(debug=False):
    nc = bacc.Bacc(
        "TRN2", target_bir_lowering=False, debug=debug, num_devices=NCORES
    )
    # Inputs arrive pre-tiled from the host: [t, h, p, j, c] =
    # transposed_tensor[(2t+j)*128 + p, h*HW + c], so each (t, h) strip is
    # one fully contiguous [128, 2, HW] DMA (2KB per partition).
    HW_ = NF // 2
    xT = nc.dram_tensor(
        "xT", [K // (2 * P), 2, P, 2, HW_], FP8, kind="ExternalInput"
    ).ap()
    wT = nc.dram_tensor(
        "wT", [K // (2 * P), 2, P, 2, HW_], FP8, kind="ExternalInput"
    ).ap()
    bias = nc.dram_tensor("bias", [NF], F32, kind="ExternalInput").ap()
    outT = nc.dram_tensor("outT", [NF, MS], F32, kind="ExternalOutput").ap()

    U16 = mybir.dt.uint16
    NG = 2  # n-tiles per group; NG*MT psum banks live at once

    with tile.TileContext(nc) as tc:
        with ExitStack() as ctx:
            const = ctx.enter_context(tc.tile_pool(name="const", bufs=1))
            res = ctx.enter_context(tc.tile_pool(name="res", bufs=1))
            psum = ctx.enter_context(
                tc.tile_pool(name="ps", bufs=1, space=bass.MemorySpace.PSUM)
            )
            outp = ctx.enter_context(tc.tile_pool(name="out", bufs=3))

            # bias[n] laid out [128, 16]: column t holds bias[t*128:(t+1)*128]
            bias_t = const.tile([P, NT], F32)
            nc.gpsimd.dma_start(
                out=bias_t[:], in_=bias.rearrange("(t p) -> p t", p=P)
            )

            HW = NF // 2  # half-strip width

            def load_bin(dram, name, t, h, dma_eng):
                # DoubleRow operand strip [128, 2, HW] fp8, contiguous DMA.
                # Binarize in place: keep sign bit, force the rest to 1.0.
                # The pass runs on a uint16 view (two fp8 per ALU element).
                tl = res.tile([P, 2, HW], FP8, tag=f"{name}{t}_{h}")
                dma_eng.dma_start(out=tl[:], in_=dram[t, h])
                nc.vector.tensor_scalar(
                    tl[:].bitcast(U16),
                    tl[:].bitcast(U16),
                    0x8080,
                    0x3838,
                    mybir.AluOpType.bitwise_and,
                    mybir.AluOpType.bitwise_or,
                )
                return tl

            # x: both halves needed by every group -> interleave by strip.
            # w: half 1 only feeds n-tiles 8..15 (groups 4+) -> defer all
            # of it behind half 0 so early HBM bandwidth unblocks group 0.
            wb = [[None, None] for _ in range(KT2)]
            xb = []
            for t in range(KT2):
                wb[t][0] = load_bin(wT, "w", t, 0, nc.sync)
                xb.append(
                    [
                        load_bin(xT, "x", t, 0, nc.scalar),
                        load_bin(xT, "x", t, 1, nc.scalar),
                    ]
                )
            for t in range(KT2):
                wb[t][1] = load_bin(wT, "w", t, 1, nc.sync)

            NPH = HW // P   # n-tiles per w half
            MCH = HW // MC  # m-chunks per x half

            def w_slice(t, n):
                return wb[t][n // NPH][
                    :, :, (n % NPH) * P : (n % NPH + 1) * P
                ]

            def x_slice(t, mc):
                return xb[t][mc // MCH][
                    :, :, (mc % MCH) * MC : (mc % MCH + 1) * MC
                ]

            NGRP = NT // NG
            for g in range(NGRP):
                pss = [
                    [
                        psum.tile(
                            [P, MC], F32, tag=f"ps{i}_{mc}", name=f"ps_{g}_{i}_{mc}"
                        )
                        for mc in range(MT)
                    ]
                    for i in range(NG)
                ]
                ots = [
                    outp.tile([P, MS], F32, tag=f"o{i}", name=f"o_{g}_{i}")
                    for i in range(NG)
                ]

                def evacuate(i, mc):
                    n = g * NG + i
                    nc.scalar.activation(
                        ots[i][:, mc * MC : (mc + 1) * MC],
                        pss[i][mc][:],
                        mybir.ActivationFunctionType.Identity,
                        bias=bias_t[:, n : n + 1],
                    )
                    nc.sync.dma_start(
                        out=outT[n * P : (n + 1) * P, mc * MC : (mc + 1) * MC],
                        in_=ots[i][:, mc * MC : (mc + 1) * MC],
                    )

                if g < NGRP - 1:
                    # k-tile outer: consume input strips as they stream in.
                    for t in range(KT2):
                        for i in range(NG):
                            for mc in range(MT):
                                nc.tensor.matmul(
                                    pss[i][mc][:],
                                    w_slice(t, g * NG + i),
                                    x_slice(t, mc),
                                    start=(t == 0),
                                    stop=(t == KT2 - 1),
                                    perf_mode=mybir.MatmulPerfMode.DoubleRow,
                                )
                    for i in range(NG):
                        for mc in range(MT):
                            evacuate(i, mc)
                else:
                    # Last group: bank-major so evacuation and output DMA of
                    # bank b overlap the matmuls of bank b+1 (shrinks the
                    # kernel tail to one bank's epilogue).
                    for i in range(NG):
                        for mc in range(MT):
                            for t in range(KT2):
                                nc.tensor.matmul(
                                    pss[i][mc][:],
                                    w_slice(t, g * NG + i),
                                    x_slice(t, mc),
                                    start=(t == 0),
                                    stop=(t == KT2 - 1),
                                    perf_mode=mybir.MatmulPerfMode.DoubleRow,
                                )
                            evacuate(i, mc)

    nc.compile()
    return nc


_NC = None


def _get_nc():
    global _NC
    if _NC is None:
        _NC = build_nc()
    return _NC


def _to_fp8_T(a):
    # Transposed fp8 copy preserving the SIGN of every element exactly
    # (magnitudes are irrelevant downstream -- the device binarizes).
    # Magnitudes are clipped into e4m3 range so the cast can't flush to
    # zero or overflow, and exact zeros are encoded as tiny NEGATIVES
    # because reference binarize maps 0 -> -1.  The result is pre-tiled to
    # [t, h, p, j, c] so each device strip is one contiguous DMA.
    at = a.T
    mag = np.clip(np.abs(at), 0.002, 240.0)
    enc = np.where(at > 0, mag, -mag).astype(ml_dtypes.float8_e4m3fn)
    kk, cols = enc.shape
    tiled = enc.reshape(kk // (2 * P), 2, P, 2, cols // 2).transpose(0, 3, 2, 1, 4)
    return np.ascontiguousarray(tiled)


def make_in_maps(x, weight, bias):
    x = np.asarray(x, dtype=np.float32)
    weight = np.asarray(weight, dtype=np.float32)
    bias = np.asarray(bias, dtype=np.float32)
    wTb = _to_fp8_T(weight)
    in_maps = []
    for i in range(NCORES):
        xTb = _to_fp8_T(x[i * MS : (i + 1) * MS, :])
        in_maps.append({"xT": xTb, "wT": wTb, "bias": bias})
    return in_maps


def assemble_out(results):
    out = np.empty((MTOT, NF), dtype=np.float32)
    for i in range(NCORES):
        out[i * MS : (i + 1) * MS, :] = results[i]["outT"].T
    return out


def run(x, weight, bias, trace=False, **kwargs):
    nc = _get_nc()
    in_maps = make_in_maps(x, weight, bias)
    res = run_bass_kernel_spmd(
        nc, in_maps, list(range(NCORES)), trace=trace, **kwargs
    )
    return assemble_out(res.results), res


def kernel(x, weight, bias):
    out, _ = run(x, weight, bias)
    return out
